# revision 11
# baseline (speedup 1.0000x reference)
"""Trainium2 Bass kernel for CombinedICIRLoss (Kendall tau + ListNet + pairwise margin).

Contract: kernel(predictions, targets) takes FULL [32,1024] f32 inputs, returns the
FULL scalar loss (0-d float32 ndarray). Internally shards batch dim across 8
NeuronCores (4 samples each), runs a Bass/Tile kernel per core, and combines tiny
per-sample partial sums on the host.
"""

import numpy as np

B, N = 32, 1024
NCORES = 8
SPC = B // NCORES          # samples per core
JC = N // 128              # j-chunks per sample
KT_INV = 10.0              # 1 / KT_TEMP
NEG30 = -1.0e30
POI = -1.0e6               # poison for invalid-i entries

_cache = {}


def _patch_tile_drain():
    """This container's walrus build only accepts one semaphore wait per CTRL
    instruction; Tile's final drain attaches one wait per live semaphore.
    Split them across consecutive drains (same engine => sequential => same
    semantics)."""
    from concourse.tile import TileContext
    if getattr(TileContext, "_drainfix", False):
        return
    import bass_rust
    from concourse.vector_clock import ScopedClock

    def patched(self, tick_clock, wait_clock):
        drain_inst = self.nc.sync.drain()
        wait_clock.add_sem_waits(
            drain_inst.ins, ScopedClock({None: tick_clock.global_clock})
        )
        ins = drain_inst.ins
        si = ins.sync_info
        if si is not None and len(si.on_wait) > 1:
            waits = list(si.on_wait)
            ins.sync_info = bass_rust.SyncInfo(
                on_wait=waits[:1], on_update=list(si.on_update)
            )
            for w in waits[1:]:
                d2 = self.nc.sync.drain()
                d2.ins.sync_info = bass_rust.SyncInfo(on_wait=[w], on_update=[])
        self.nc.all_engine_barrier()
        popped = self.nc._tile_sem_poison_stack.pop()
        assert popped is self._sem_poison
        self.nc.clear_and_free_semaphores(list(self.sems.allocated().values()))
        self.nc.all_engine_barrier()

    TileContext._drain_and_barrier = patched
    TileContext._drainfix = True


def _split_multi_waits(nc):
    """This walrus build accepts only one semaphore wait per instruction.
    Hoist extra waits onto single-wait NoOps inserted just before, on the same
    engine (same stream position => identical semantics)."""
    import concourse.mybir as mybir
    import bass_rust

    cnt = 0
    for f in nc.m.functions:
        for bb in f.blocks:
            changed = False
            out = []
            for ins in bb.instructions:
                si = ins.sync_info
                if si is not None and len(si.on_wait) > 1:
                    waits = list(si.on_wait)
                    for w in waits[:-1]:
                        cnt += 1
                        nop = mybir.InstNoOp(
                            name=f"waitfix-{cnt}",
                            engine=ins.engine,
                            sync_info=bass_rust.SyncInfo(on_wait=[w], on_update=[]),
                        )
                        out.append(nop)
                    ins.sync_info = bass_rust.SyncInfo(
                        on_wait=[waits[-1]], on_update=list(si.on_update)
                    )
                    changed = True
                out.append(ins)
            if changed:
                bb.instructions = out
    return cnt


def _build():
    """Build the per-core Bass module: inputs p,t [4,1024] f32, output
    partials [4,4] f32 = per-sample [conc2, Mv, kl, n_valid]."""
    if "nc" in _cache:
        return _cache["nc"]
    from contextlib import ExitStack
    import concourse.bass as bass
    import concourse.mybir as mybir
    from concourse.tile import TileContext

    _patch_tile_drain()

    f32 = mybir.dt.float32
    OP = mybir.AluOpType
    AF = mybir.ActivationFunctionType
    AX = mybir.AxisListType

    nc = bass.Bass("TRN2", target_bir_lowering=False, debug=False)
    p_in = nc.dram_tensor("p", [SPC, N], f32, kind="ExternalInput")
    t_in = nc.dram_tensor("t", [SPC, N], f32, kind="ExternalInput")
    out_d = nc.dram_tensor("partials", [SPC, 4], f32, kind="ExternalOutput")

    with TileContext(nc) as tc, ExitStack() as ctx:
        persist = ctx.enter_context(tc.tile_pool(name="persist", bufs=1))
        work = ctx.enter_context(tc.tile_pool(name="work", bufs=3))
        small = ctx.enter_context(tc.tile_pool(name="small", bufs=1))
        psum_bc = ctx.enter_context(tc.tile_pool(name="psum_bc", bufs=1, space="PSUM"))
        psum_k = ctx.enter_context(tc.tile_pool(name="psum_k", bufs=1, space="PSUM"))

        # ---------- setup: flat [4,1024] and partitioned [128,32] views ----------
        p4 = persist.tile([SPC, N], f32, tag="p4")
        t4 = persist.tile([SPC, N], f32, tag="t4")
        nc.sync.dma_start(out=p4[:], in_=p_in[:, :])
        nc.sync.dma_start(out=t4[:], in_=t_in[:, :])

        p_part = persist.tile([128, SPC * JC], f32, tag="p_part")
        t_part = persist.tile([128, SPC * JC], f32, tag="t_part")
        nc.sync.dma_start(out=p_part[:], in_=p_in[:, :].rearrange("s (c k) -> k (s c)", k=128))
        nc.sync.dma_start(out=t_part[:], in_=t_in[:, :].rearrange("s (c k) -> k (s c)", k=128))

        v4 = persist.tile([SPC, N], f32, tag="v4")
        nc.vector.tensor_tensor(v4[:], t4[:], t4[:], OP.is_equal)  # NaN != NaN -> 0
        vm4 = persist.tile([SPC, N], mybir.dt.uint32, tag="vm4")
        nc.vector.tensor_tensor(vm4[:], t4[:], t4[:], OP.is_equal)
        nval = small.tile([SPC, 1], f32, tag="nval")
        nc.vector.reduce_sum(nval[:], v4[:], axis=AX.X)

        negpoi4 = persist.tile([SPC, N], f32, tag="negpoi4")
        nc.vector.memset(negpoi4[:], POI)
        ppoi4 = persist.tile([SPC, N], f32, tag="ppoi4")
        nc.vector.select(ppoi4[:], vm4[:], p4[:], negpoi4[:])
        tpoi4 = persist.tile([SPC, N], f32, tag="tpoi4")
        nc.vector.select(tpoi4[:], vm4[:], t4[:], negpoi4[:])

        v_part = persist.tile([128, SPC * JC], f32, tag="v_part")
        nc.vector.tensor_tensor(v_part[:], t_part[:], t_part[:], OP.is_equal)
        vm_part = persist.tile([128, SPC * JC], mybir.dt.uint32, tag="vm_part")
        nc.vector.tensor_tensor(vm_part[:], t_part[:], t_part[:], OP.is_equal)
        zeros_part = persist.tile([128, SPC * JC], f32, tag="zeros_part")
        nc.vector.memset(zeros_part[:], 0.0)
        ts_part = persist.tile([128, SPC * JC], f32, tag="ts_part")  # t_safe, j-layout
        nc.vector.select(ts_part[:], vm_part[:], t_part[:], zeros_part[:])
        p10 = persist.tile([128, SPC * JC], f32, tag="p10")
        nc.vector.tensor_scalar(p10[:], p_part[:], KT_INV, None, OP.mult)
        t10 = persist.tile([128, SPC * JC], f32, tag="t10")
        nc.vector.tensor_scalar(t10[:], ts_part[:], KT_INV, None, OP.mult)
        negt = persist.tile([128, SPC * JC], f32, tag="negt")
        nc.vector.tensor_scalar(negt[:], ts_part[:], -1.0, None, OP.mult)

        # mask-selector stationary for the K reduction: for tile c (sample s),
        # cols [4c..4c+4) are all zero except col 4c+s = v_part[:, c]
        vsel = persist.tile([128, 4 * SPC * JC], f32, tag="vsel")
        nc.vector.memset(vsel[:], 0.0)
        for c in range(SPC * JC):
            s = c // JC
            nc.vector.tensor_copy(vsel[:, 4 * c + s : 4 * c + s + 1], v_part[:, c : c + 1])

        ones1 = persist.tile([1, 128], f32, tag="ones1")
        nc.vector.memset(ones1[:], 1.0)
        ones_col = persist.tile([128, 1], f32, tag="ones_col")
        nc.vector.memset(ones_col[:], 1.0)

        mincol = persist.tile([128, SPC * JC], f32, tag="mincol")
        nc.vector.memset(mincol[:], 0.0)

        # per-sample poisoned rows at base partition 0 (PE operands must start
        # at partition 0/32/64; ppoi4[s:s+1] sits at partition s)
        prows, trows = [], []
        for s in range(SPC):
            pr = persist.tile([1, N], f32, tag=f"prow{s}")
            tr = persist.tile([1, N], f32, tag=f"trow{s}")
            nc.sync.dma_start(out=pr[:], in_=ppoi4[s : s + 1, :])
            nc.sync.dma_start(out=tr[:], in_=tpoi4[s : s + 1, :])
            prows.append(pr)
            trows.append(tr)

        K4 = psum_k.tile([SPC, N], f32, tag="K4")

        # ---------- main O(N^2) loop ----------
        H = N // 2
        for s in range(SPC):
            # broadcast rows (poisoned p and t of sample s) across 128 partitions via PE
            pb = psum_bc.tile([128, N], f32, tag="pb")
            tb = psum_bc.tile([128, N], f32, tag="tb")
            for h in range(2):
                sl = slice(h * H, (h + 1) * H)
                nc.tensor.matmul(pb[:, sl], ones1[0:1, :], prows[s][0:1, sl],
                                 start=True, stop=True)
                nc.tensor.matmul(tb[:, sl], ones1[0:1, :], trows[s][0:1, sl],
                                 start=True, stop=True)
            for jc in range(JC):
                c = s * JC + jc
                ps_t = work.tile([128, N], f32, tag="ps")
                nc.scalar.activation(ps_t[:], pb[:], AF.Tanh,
                                     bias=p10[:, c : c + 1], scale=-KT_INV)
                ts_t = work.tile([128, N], f32, tag="ts")
                nc.scalar.activation(ts_t[:], tb[:], AF.Tanh,
                                     bias=t10[:, c : c + 1], scale=-KT_INV)
                sg_t = work.tile([128, N], f32, tag="sg")
                # sign(t_i_poi - t_j): +1 on poisoned-invalid i jointly with
                # (p_i_poi - p_j) ~ -1e6 makes q = +1e6 -> min(q,1)=1 -> h=0
                nc.scalar.activation(sg_t[:], tb[:], AF.Sign,
                                     bias=negt[:, c : c + 1], scale=1.0)
                z_t = work.tile([128, N], f32, tag="z")
                nc.vector.tensor_tensor(z_t[:], ps_t[:], ts_t[:], OP.mult)
                for h in range(2):
                    sl = slice(h * H, (h + 1) * H)
                    nc.tensor.matmul(K4[:, sl], vsel[:, 4 * c : 4 * c + 4],
                                     z_t[:, sl], start=(c == 0), stop=(c == SPC * JC - 1))
                nq_t = work.tile([128, N], f32, tag="nq")
                # q = (p_i_poi - p_j) * sign(t_i_poi - t_j) = pd * sign(td)
                nc.vector.scalar_tensor_tensor(nq_t[:], pb[:], p_part[:, c : c + 1],
                                               sg_t[:], OP.subtract, OP.mult)
                mq_t = work.tile([128, N], f32, tag="mq")
                # out = min(q,1); accum (reduce op1=add, init scalar2=0) = sum_i min(q,1)
                nc.vector.tensor_scalar(mq_t[:], nq_t[:], 1.0, 0.0, OP.min, OP.add,
                                        accum_out=mincol[:, c : c + 1])

        # ---------- pairwise-margin tail: Mv[s] = sum_j v_j * mincol_j ----------
        mr4 = persist.tile([128, SPC], f32, tag="mr4")
        junk8 = persist.tile([128, JC], f32, tag="junk8")
        for s in range(SPC):
            nc.vector.tensor_tensor(
                junk8[:], mincol[:, s * JC : (s + 1) * JC],
                v_part[:, s * JC : (s + 1) * JC], OP.mult)
            nc.vector.reduce_sum(mr4[:, s : s + 1], junk8[:], axis=AX.X)
        Msum = psum_k.tile([SPC, 1], f32, tag="Msum")
        nc.tensor.matmul(Msum[:], mr4[:, 0:SPC], ones_col[:], start=True, stop=True)

        # ---------- Kendall tail: conc2[s] = sum_i v_i * K4[s,i] ----------
        kv = small.tile([SPC, N], f32, tag="kv")
        nc.vector.tensor_tensor(kv[:], K4[:], v4[:], OP.mult)
        conc2 = small.tile([SPC, 1], f32, tag="conc2")
        nc.vector.reduce_sum(conc2[:], kv[:], axis=AX.X)

        # ---------- ListNet ----------
        neg30 = persist.tile([SPC, N], f32, tag="neg30")
        nc.vector.memset(neg30[:], NEG30)
        mp4 = small.tile([SPC, N], f32, tag="mp4")
        nc.vector.select(mp4[:], vm4[:], p4[:], neg30[:])
        mt4 = small.tile([SPC, N], f32, tag="mt4")
        nc.vector.select(mt4[:], vm4[:], t4[:], neg30[:])

        mxp = small.tile([SPC, 1], f32, tag="mxp")
        nc.vector.reduce_max(mxp[:], mp4[:], axis=AX.X)
        nmxp = small.tile([SPC, 1], f32, tag="nmxp")
        nc.vector.tensor_scalar(nmxp[:], mxp[:], -1.0, None, OP.mult)
        mxt = small.tile([SPC, 1], f32, tag="mxt")
        nc.vector.reduce_max(mxt[:], mt4[:], axis=AX.X)
        nmxt = small.tile([SPC, 1], f32, tag="nmxt")
        nc.vector.tensor_scalar(nmxt[:], mxt[:], -1.0, None, OP.mult)

        ep = small.tile([SPC, N], f32, tag="ep")
        sep = small.tile([SPC, 1], f32, tag="sep")
        nc.scalar.activation(ep[:], mp4[:], AF.Exp, bias=nmxp[:], scale=1.0,
                             accum_out=sep[:])
        et = small.tile([SPC, N], f32, tag="et")
        st4 = small.tile([SPC, 1], f32, tag="st4")
        nc.scalar.activation(et[:], mt4[:], AF.Exp, bias=nmxt[:], scale=1.0,
                             accum_out=st4[:])
        lnp = small.tile([SPC, 1], f32, tag="lnp")
        nc.scalar.activation(lnp[:], sep[:], AF.Ln)
        lnt = small.tile([SPC, 1], f32, tag="lnt")
        nc.scalar.activation(lnt[:], st4[:], AF.Ln)

        # sh = (mxp + lnp) - (mxt + lnt)
        sh1 = small.tile([SPC, 1], f32, tag="sh1")
        nc.vector.tensor_tensor(sh1[:], mxp[:], mxt[:], OP.subtract)
        sh2 = small.tile([SPC, 1], f32, tag="sh2")
        nc.vector.tensor_tensor(sh2[:], lnp[:], lnt[:], OP.subtract)
        sh = small.tile([SPC, 1], f32, tag="sh")
        nc.vector.tensor_tensor(sh[:], sh1[:], sh2[:], OP.add)

        d4 = small.tile([SPC, N], f32, tag="d4")
        nc.vector.tensor_tensor(d4[:], mt4[:], mp4[:], OP.subtract)
        d4b = small.tile([SPC, N], f32, tag="d4b")
        nc.vector.tensor_scalar(d4b[:], d4[:], sh[:], None, OP.add)
        w4 = small.tile([SPC, N], f32, tag="w4")
        nc.vector.tensor_tensor(w4[:], et[:], d4b[:], OP.mult)
        r4 = small.tile([SPC, 1], f32, tag="r4")
        nc.vector.reduce_sum(r4[:], w4[:], axis=AX.X)
        rst = small.tile([SPC, 1], f32, tag="rst")
        nc.vector.reciprocal(rst[:], st4[:])
        kl4 = small.tile([SPC, 1], f32, tag="kl4")
        nc.vector.tensor_tensor(kl4[:], r4[:], rst[:], OP.mult)

        # ---------- pack + store ----------
        outs = small.tile([SPC, 4], f32, tag="outs")
        nc.vector.tensor_copy(outs[:, 0:1], conc2[:])
        nc.vector.tensor_copy(outs[:, 1:2], Msum[:])
        nc.vector.tensor_copy(outs[:, 2:3], kl4[:])
        nc.vector.tensor_copy(outs[:, 3:4], nval[:])
        nc.sync.dma_start(out=out_d[:, :], in_=outs[:])

    _split_multi_waits(nc)
    _cache["nc"] = nc
    return nc


def _run_device(predictions, targets):
    from concourse.bass_utils import run_bass_kernel_spmd

    nc = _build()
    p = np.ascontiguousarray(predictions, dtype=np.float32)
    t = np.ascontiguousarray(targets, dtype=np.float32)
    in_maps = [
        {"p": p[c * SPC : (c + 1) * SPC], "t": t[c * SPC : (c + 1) * SPC]}
        for c in range(NCORES)
    ]
    res = run_bass_kernel_spmd(nc, in_maps, core_ids=list(range(NCORES)))
    return np.concatenate([res.results[c]["partials"] for c in range(NCORES)], axis=0)


def _combine(partials):
    """partials [B,4] f64-able: cols conc2, Mv, kl, n_valid -> scalar loss."""
    pa = partials.astype(np.float64)
    conc2, Mv, kl, n = pa[:, 0], pa[:, 1], pa[:, 2], pa[:, 3]
    ok = n > 1
    n_ok = max(int(ok.sum()), 1)
    tri = np.maximum(n * (n - 1) / 2.0, 1.0)
    conc = (conc2 / 2.0) / tri
    pw_num = 1024.0 * n - Mv - n
    pw_den = np.maximum(n * (n - 1), 1.0)
    pw = pw_num / pw_den
    kendall = -np.sum(np.where(ok, conc, 0.0)) / n_ok
    listnet = np.sum(np.where(ok, kl, 0.0)) / n_ok
    pairwise = np.sum(np.where(ok, pw, 0.0)) / n_ok
    return np.float32(kendall + listnet + pairwise)


def kernel(predictions, targets):
    partials = _run_device(predictions, targets)
    return np.asarray(_combine(partials), dtype=np.float32)


def estimate_ns():
    """Cost-model (TimelineSim) single-core duration estimate in ns."""
    from concourse.timeline_sim import TimelineSim

    nc = _build()
    sim = TimelineSim(nc)
    return sim.simulate()
